# revision 6
# baseline (speedup 1.0000x reference)
"""Trainium2 Bass kernel for a 2-layer GAT (PyG GATConv semantics, eval mode).

v2 design (8 NeuronCores, SPMD, dst-sharded nodes / edge gather by src):
  - Nodes sharded by dst owner; per-layer node tables are AllGathered as
    bf16 rows of 256B: [h(bf16) | a_src(f32)].  A core-local a_dst region is
    appended to the same DRAM tensor (rows TROWS..TROWS+NPCP) so the last
    int16-reach chunk of the gather covers both the table tail and all a_dst
    rows -> one gather call instead of two.
  - Edges routed to dst owner, bucketed by (128-dst block, 32767-row chunk),
    each bucket padded to a multiple of 128 slots (index-0 padding).  Buckets
    are concatenated into large per-(group, chunk) gather calls (~90 tiles)
    to amortize the ~1us fixed SWDGE cost per call; the per-bucket tile
    counts are a compile-time "plan" derived from the actual edge
    distribution (max over cores), so the kernel recompiles if the
    distribution changes shape.
  - Per 128-slot tile: mask[p, n] = (dst_local == n) in bf16; messages
    S = [w | w*h] in bf16 with w = exp(leaky_relu(a_src + a_dst)) computed in
    f32.  Aggregation is a bf16 mask-matmul accumulating [sum_w | sum_wh]
    into PSUM per 128-node block (softmax shift is unnecessary: |e| is small
    and softmax is shift-invariant).
  - Block epilogue: divide, bias, ELU, then layer-2 node rows (h2 | a_src2 |
    a_dst2) via an augmented matmul; second AllGather; same edge pipeline
    with 1 head / 40 channels; log_softmax; output.
"""

import sys

if "/opt/trn_rl_repo" not in sys.path:
    sys.path.insert(0, "/opt/trn_rl_repo")

import hashlib
from dataclasses import dataclass, field

import numpy as np
import ml_dtypes

import concourse.bass as bass
import concourse.bacc as bacc
import concourse.tile as tile
import concourse.mybir as mybir
from concourse.masks import make_identity

F32 = mybir.dt.float32
BF16 = mybir.dt.bfloat16
I16 = mybir.dt.int16
I32 = mybir.dt.int32

NEG_SLOPE = 0.2
CH = 32767  # int16 gather index reach (rows per chunk)


@dataclass(frozen=True)
class Cfg:
    N: int = 100000
    F: int = 128
    H1: int = 8
    C1: int = 8
    D2: int = 40
    NC: int = 8
    TGMAX: int = 88  # max slot-tiles per group
    NBMAX: int = 7  # max blocks per group (PSUM strip limit)
    SUBT: int = 24  # slot-tiles per gather sub-call
    ADIND: bool = False  # indirect-DMA a_dst: broken on HW (vector dyn offsets)
    dbg: bool = False

    @property
    def D1(self):
        return self.H1 * self.C1  # 64

    @property
    def NPC(self):
        return self.N // self.NC  # 12500

    @property
    def NB(self):
        return (self.NPC + 127) // 128  # 98

    @property
    def NPCP(self):
        return self.NB * 128  # 12544

    @property
    def TROWS(self):
        return self.NC * self.NPCP  # 100352

    @property
    def NK(self):
        return (self.TROWS + CH - 1) // CH  # 4


@dataclass
class Plan:
    tiles: np.ndarray  # [NB, NK] int32, slot-tiles per (block, chunk)
    groups: list  # list of lists of block ids

    def key(self):
        h = hashlib.sha256(self.tiles.tobytes())
        h.update(str(self.groups).encode())
        return h.hexdigest()


def make_plan(dst_all, trow_all, cfg: Cfg):
    """Per-(block, chunk) tile counts = max over cores, then grouped."""
    c = cfg
    counts = np.zeros((c.NC, c.NB, c.NK), np.int64)
    blk_all = (dst_all % c.NPC) // 128
    chunk_all = trow_all // CH
    core_all = dst_all // c.NPC
    key = ((core_all * c.NB) + blk_all) * c.NK + chunk_all
    cnt = np.bincount(key, minlength=c.NC * c.NB * c.NK)
    counts = cnt.reshape(c.NC, c.NB, c.NK)
    tiles = ((counts.max(axis=0) + 127) // 128).astype(np.int32)  # [NB, NK]

    groups = []
    cur, cur_t = [], 0
    for b in range(c.NB):
        t = int(tiles[b].sum())
        if cur and (cur_t + t > c.TGMAX or len(cur) >= c.NBMAX):
            groups.append(cur)
            cur, cur_t = [], 0
        cur.append(b)
        cur_t += t
    if cur:
        groups.append(cur)
    return Plan(tiles=tiles, groups=groups)


class GroupLayout:
    """Compile-time layout of one group's gather calls and slot tiles."""

    def __init__(self, plan: Plan, cfg: Cfg, blocks):
        c = cfg
        self.blocks = blocks
        t = plan.tiles
        self.Tk = [int(sum(t[b][k] for b in blocks)) for k in range(c.NK)]
        self.TG = sum(self.Tk)
        self.T012 = sum(self.Tk[: c.NK - 1])
        self.T3 = self.Tk[c.NK - 1]
        # group-slot tile offset of (b, k) bucket
        self.off = {}
        pos = 0
        for k in range(c.NK):
            for b in blocks:
                self.off[(b, k)] = pos
                pos += int(t[b][k])
        # per-block list of (start_tile, ntiles) runs in ascending tile order
        self.runs = {
            b: [(self.off[(b, k)], int(t[b][k])) for k in range(c.NK) if t[b][k]]
            for b in blocks
        }
        # idx widths (positions) per call: NK table calls then one ad call
        self.call_pos = [self.Tk[k] * 128 for k in range(c.NK)] + [self.TG * 128]


def layouts(plan: Plan, cfg: Cfg):
    return [GroupLayout(plan, cfg, blocks) for blocks in plan.groups]


# ---------------------------------------------------------------- host side


def wrap_idx(flat):
    """[N] int -> [128, N//16] int16 wrapped in 16 partitions, replicated."""
    w = flat.reshape(-1, 16).T.astype(np.int16)  # [16, N/16]
    return np.tile(w, (8, 1))


def preprocess(x, edge_index, cfg: Cfg):
    c = cfg
    src = np.concatenate([np.asarray(edge_index[0]), np.arange(c.N)]).astype(np.int64)
    dst = np.concatenate([np.asarray(edge_index[1]), np.arange(c.N)]).astype(np.int64)
    trow = (src // c.NPC) * c.NPCP + (src % c.NPC)

    plan = make_plan(dst, trow, c)
    lys = layouts(plan, c)
    TGSUM = sum(ly.TG for ly in lys)
    IPOS = sum(sum(ly.call_pos) for ly in lys)  # total idx positions

    in_maps = []
    for core in range(c.NC):
        lo, hi = core * c.NPC, (core + 1) * c.NPC
        m = (dst >= lo) & (dst < hi)
        s_t, d_l = trow[m], (dst[m] - lo)
        blk = d_l // 128
        chunk = s_t // CH
        order = np.lexsort((s_t, chunk, blk))
        s_t, d_l, blk, chunk = s_t[order], d_l[order], blk[order], chunk[order]
        key = blk * c.NK + chunk
        uniq, start, cnt = np.unique(key, return_index=True, return_counts=True)
        bucket = {}
        for u, st, n in zip(uniq, start, cnt):
            b, k = divmod(int(u), c.NK)
            bucket[(b, k)] = (st, int(n))

        gidx = np.zeros(IPOS, np.int64)
        oad = np.zeros((128, TGSUM), np.int32)
        dstib = np.full((128, TGSUM), 584.0, np.float32)

        ipos = 0  # running idx position
        gbase = 0  # running group tile base
        for ly in lys:
            # slot-tile fill (also reused for ad index build)
            slot_dl = np.zeros(ly.TG * 128, np.int64)  # local dst per slot
            slot_valid = np.zeros(ly.TG * 128, bool)
            for k in range(c.NK):
                for b in ly.blocks:
                    nt = int(plan.tiles[b][k])
                    if nt == 0:
                        continue
                    st, n = bucket.get((b, k), (0, 0))
                    t0 = ly.off[(b, k)]
                    if n:
                        sl = slice(t0 * 128, t0 * 128 + n)
                        slot_dl[sl] = d_l[st : st + n]
                        slot_valid[sl] = True
            # table idx per chunk call
            for k in range(c.NK):
                base = k * CH
                w = np.zeros(ly.call_pos[k], np.int64)
                pos0 = ly.off[(ly.blocks[0], k)] * 128  # first tile of chunk k
                for b in ly.blocks:
                    nt = int(plan.tiles[b][k])
                    if nt == 0:
                        continue
                    st, n = bucket.get((b, k), (0, 0))
                    t0 = ly.off[(b, k)] * 128 - pos0
                    if n:
                        w[t0 : t0 + n] = s_t[st : st + n] - base
                gidx[ipos : ipos + ly.call_pos[k]] = w
                ipos += ly.call_pos[k]
            # ad idx call: local dst row per slot
            w = np.where(slot_valid, slot_dl, 0)
            gidx[ipos : ipos + ly.TG * 128] = w
            oad[:, gbase : gbase + ly.TG] = w.reshape(ly.TG, 128).T
            ipos += ly.TG * 128
            # dstib
            dloc = np.where(slot_valid, slot_dl % 128, 584).reshape(ly.TG, 128)
            dstib[:, gbase : gbase + ly.TG] = dloc.T
            gbase += ly.TG

        xs = np.zeros((c.NPCP, c.F), np.float32)
        xs[: c.NPC] = np.asarray(x)[lo:hi]
        in_maps.append(
            {
                "xT": np.ascontiguousarray(xs.T),
                "gidx": wrap_idx(gidx),
                "oad": oad,
                "dstib": dstib.astype(ml_dtypes.bfloat16),
                "iotab": np.tile(
                    np.arange(128, dtype=np.float32), (128, 1)
                ).astype(ml_dtypes.bfloat16),
                "bmask": np.kron(np.eye(8), np.ones((8, 1))).astype(np.float32),
            }
        )
    return plan, in_maps


# -------------------------------------------------------------- device side


def build(nc, cfg: Cfg, plan: Plan, repeats: int = 1):
    c = cfg
    D1, D2, H1 = c.D1, c.D2, c.H1
    lys = layouts(plan, c)
    TGSUM = sum(ly.TG for ly in lys)
    IPOS = sum(sum(ly.call_pos) for ly in lys)

    xT_t = nc.dram_tensor("xT", [c.F, c.NPCP], F32, kind="ExternalInput")
    W1 = nc.dram_tensor("W1", [c.F, D1], F32, kind="ExternalInput")
    att_src1 = nc.dram_tensor("att_src1", [H1, c.C1], F32, kind="ExternalInput")
    att_dst1 = nc.dram_tensor("att_dst1", [H1, c.C1], F32, kind="ExternalInput")
    b1 = nc.dram_tensor("b1", [D1], F32, kind="ExternalInput")
    W2 = nc.dram_tensor("W2", [D1, D2], F32, kind="ExternalInput")
    att_src2 = nc.dram_tensor("att_src2", [1, D2], F32, kind="ExternalInput")
    att_dst2 = nc.dram_tensor("att_dst2", [1, D2], F32, kind="ExternalInput")
    b2 = nc.dram_tensor("b2", [D2], F32, kind="ExternalInput")
    gidx_t = nc.dram_tensor("gidx", [128, IPOS // 16], I16, kind="ExternalInput")
    dstib_t = nc.dram_tensor("dstib", [128, TGSUM], BF16, kind="ExternalInput")
    oad_t = nc.dram_tensor("oad", [128, TGSUM], I32, kind="ExternalInput")
    iotab_t = nc.dram_tensor("iotab", [128, 128], BF16, kind="ExternalInput")
    bmask_t = nc.dram_tensor("bmask", [64, 8], F32, kind="ExternalInput")
    out_t = nc.dram_tensor("out", [c.NPCP, D2], F32, kind="ExternalOutput")

    with tile.TileContext(nc) as tc:
        with (
            tc.tile_pool(name="dram", bufs=1, space="DRAM") as dram,
            tc.tile_pool(name="const", bufs=1) as cst,
            tc.tile_pool(name="wk", bufs=2) as wk,
            tc.tile_pool(name="gsrc", bufs=2) as gsp,
            tc.tile_pool(name="gc3", bufs=2) as gcp,
            tc.tile_pool(name="msk", bufs=2) as mkp,
            tc.tile_pool(name="sS", bufs=2) as ssp,
            tc.tile_pool(name="psA", bufs=2, space="PSUM") as psA,
            tc.tile_pool(name="psB", bufs=2, space="PSUM") as psB,
        ):
            # ---- constants
            ident = cst.tile([128, 128], F32)
            make_identity(nc, ident[:])
            iota_b = cst.tile([128, 128], BF16)
            nc.sync.dma_start(out=iota_b[:], in_=iotab_t.ap())

            w1sb = cst.tile([c.F, D1], F32)
            nc.sync.dma_start(out=w1sb[:], in_=W1.ap())
            w1T_ps = psA.tile([D1, c.F], F32, tag="pT")
            nc.tensor.transpose(out=w1T_ps[:], in_=w1sb[:], identity=ident[:])
            w1T = cst.tile([D1, c.F], F32)
            nc.vector.tensor_copy(out=w1T[:], in_=w1T_ps[:])

            blockmask = cst.tile([D1, H1], F32)
            nc.sync.dma_start(out=blockmask[:], in_=bmask_t.ap())
            atts_c = cst.tile([D1, 2], F32)
            nc.sync.dma_start(
                out=atts_c[:, 0:1], in_=att_src1.ap().rearrange("h c -> (h c)")[:, None]
            )
            nc.sync.dma_start(
                out=atts_c[:, 1:2], in_=att_dst1.ap().rearrange("h c -> (h c)")[:, None]
            )
            ablk = cst.tile([D1, 2 * H1], F32)
            nc.vector.tensor_tensor(
                out=ablk[:, 0:H1],
                in0=atts_c[:, 0:1].to_broadcast([D1, H1]),
                in1=blockmask[:],
                op=mybir.AluOpType.mult,
            )
            nc.vector.tensor_tensor(
                out=ablk[:, H1 : 2 * H1],
                in0=atts_c[:, 1:2].to_broadcast([D1, H1]),
                in1=blockmask[:],
                op=mybir.AluOpType.mult,
            )
            acols_ps = psA.tile([c.F, 2 * H1], F32, tag="pT")
            nc.tensor.matmul(
                out=acols_ps[:], lhsT=w1T[:], rhs=ablk[:], start=True, stop=True
            )
            w1aug = cst.tile([c.F, D1 + 2 * H1], F32)
            nc.vector.tensor_copy(out=w1aug[:, 0:D1], in_=w1sb[:])
            nc.vector.tensor_copy(out=w1aug[:, D1 : D1 + 2 * H1], in_=acols_ps[:])

            w2sb = cst.tile([D1, D2], F32)
            nc.sync.dma_start(out=w2sb[:], in_=W2.ap())
            w2T_ps = psA.tile([D2, D1], F32, tag="pT")
            nc.tensor.transpose(out=w2T_ps[:], in_=w2sb[:], identity=ident[0:D1, 0:D1])
            w2T = cst.tile([D2, D1], F32)
            nc.vector.tensor_copy(out=w2T[:], in_=w2T_ps[:])
            att2 = cst.tile([D2, 2], F32)
            nc.sync.dma_start(
                out=att2[:, 0:1], in_=att_src2.ap().rearrange("o c -> (o c)")[:, None]
            )
            nc.sync.dma_start(
                out=att2[:, 1:2], in_=att_dst2.ap().rearrange("o c -> (o c)")[:, None]
            )
            v2_ps = psA.tile([D1, 2], F32, tag="pT")
            nc.tensor.matmul(out=v2_ps[:], lhsT=w2T[:], rhs=att2[:], start=True, stop=True)
            w2aug = cst.tile([D1, D2 + 2], F32)
            nc.vector.tensor_copy(out=w2aug[:, 0:D2], in_=w2sb[:])
            nc.vector.tensor_copy(out=w2aug[:, D2 : D2 + 2], in_=v2_ps[:])

            ones1 = cst.tile([1, 128], F32)
            nc.vector.memset(ones1[:], 1.0)
            b1row = cst.tile([1, D1], F32)
            nc.sync.dma_start(out=b1row[:], in_=b1.ap()[None, :])
            b1rep_ps = psA.tile([128, D1], F32, tag="pT")
            nc.tensor.matmul(out=b1rep_ps[:], lhsT=ones1[:], rhs=b1row[:], start=True, stop=True)
            b1rep = cst.tile([128, D1], F32)
            nc.vector.tensor_copy(out=b1rep[:], in_=b1rep_ps[:])
            b2row = cst.tile([1, D2], F32)
            nc.sync.dma_start(out=b2row[:], in_=b2.ap()[None, :])
            b2rep_ps = psA.tile([128, D2], F32, tag="pT")
            nc.tensor.matmul(out=b2rep_ps[:], lhsT=ones1[:], rhs=b2row[:], start=True, stop=True)
            b2rep = cst.tile([128, D2], F32)
            nc.vector.tensor_copy(out=b2rep[:], in_=b2rep_ps[:])

            for _rep in range(repeats):
                slab1 = dram.tile([c.NPCP, 128], BF16, tag=f"slab1_{_rep}")
                comb1 = dram.tile(
                    [c.TROWS, 128], BF16, addr_space="Shared", tag=f"comb1_{_rep}"
                )
                adl1 = dram.tile([c.NPCP, 8 if c.ADIND else 64], F32, tag=f"adl1_{_rep}")
                slab2 = dram.tile([c.NPCP, 128], BF16, tag=f"slab2_{_rep}")
                comb2 = dram.tile(
                    [c.TROWS, 128], BF16, addr_space="Shared", tag=f"comb2_{_rep}"
                )
                adl2 = dram.tile([c.NPCP, 8 if c.ADIND else 64], F32, tag=f"adl2_{_rep}")

                # ---- phase A: h1 | a_src1 | a_dst1 per node tile
                with nc.named_scope("phaseA"):
                    for t in range(c.NB):
                        xT = wk.tile([128, 128], F32, tag="xT")
                        nc.sync.dma_start(
                            out=xT[:], in_=xT_t.ap()[:, t * 128 : (t + 1) * 128]
                        )
                        h_ps = psB.tile([128, D1 + 2 * H1], F32, tag="hps")
                        nc.tensor.matmul(
                            out=h_ps[:], lhsT=xT[:], rhs=w1aug[:], start=True, stop=True
                        )
                        s1 = wk.tile([128, 80], BF16, tag="s1")
                        nc.vector.tensor_copy(out=s1[:, 0:D1], in_=h_ps[:, 0:D1])
                        s1f = s1[:].bitcast(F32)
                        nc.vector.tensor_copy(
                            out=s1f[:, 32:40], in_=h_ps[:, D1 : D1 + H1]
                        )
                        nc.sync.dma_start(
                            out=slab1[t * 128 : (t + 1) * 128, 0:80], in_=s1[:]
                        )
                        ad1 = wk.tile([128, 8], F32, tag="ad1")
                        nc.vector.tensor_copy(
                            out=ad1[:], in_=h_ps[:, D1 + H1 : D1 + 2 * H1]
                        )
                        nc.sync.dma_start(
                            out=adl1[t * 128 : (t + 1) * 128, 0:8], in_=ad1[:]
                        )

                with nc.named_scope("ag1"):
                    nc.gpsimd.collective_compute(
                        "AllGather",
                        mybir.AluOpType.bypass,
                        replica_groups=[list(range(c.NC))],
                        ins=[slab1[:, :].opt()],
                        outs=[comb1[:, :].opt()],
                    )

                def edge_layer(layer):
                    comb = comb1 if layer == 1 else comb2
                    adl = adl1 if layer == 1 else adl2
                    NH = H1 if layer == 1 else 1
                    DV = D1 if layer == 1 else D2
                    SW = NH + DV
                    AOFF = 32 if layer == 1 else 20  # a_src f32 elem offset in row
                    ipos = 0
                    gbase = 0
                    for gi, ly in enumerate(lys):
                        TG = ly.TG
                        # index + dstib loads
                        iw = sum(ly.call_pos) // 16
                        ixt = wk.tile([128, iw], I16, tag="ixt")
                        nc.sync.dma_start(
                            out=ixt[:], in_=gidx_t.ap()[:, ipos // 16 : (ipos // 16) + iw]
                        )
                        dsb = wk.tile([128, TG], BF16, tag="dsb")
                        nc.sync.dma_start(
                            out=dsb[:], in_=dstib_t.ap()[:, gbase : gbase + TG]
                        )
                        # mask
                        mask = mkp.tile([128, TG * 128], BF16, tag="mask")
                        nc.vector.tensor_tensor(
                            out=mask[:],
                            in0=iota_b[:][:, None, :].to_broadcast([128, TG, 128]),
                            in1=dsb[:].to_broadcast([128, TG, 128]),
                            op=mybir.AluOpType.is_equal,
                        )
                        # gathers: NK table-chunk calls into one tile + 1 ad call
                        gsrc = gsp.tile([128, TG * 128], BF16, tag="gsrc")
                        gsrc3 = gsrc[:].rearrange("p (t e) -> p t e", e=128)
                        ADW = 8 if c.ADIND else 64
                        ad = gcp.tile([128, TG * ADW], F32, tag="ad")
                        ad3 = ad[:].rearrange("p (t e) -> p t e", e=ADW)
                        ioff = 0
                        toff = 0
                        qn = gi  # rotate queues across groups/sub-calls
                        for k in range(c.NK):
                            rows = min(CH, c.TROWS - k * CH)
                            for s0 in range(0, ly.Tk[k], c.SUBT):
                                nt = min(c.SUBT, ly.Tk[k] - s0)
                                npos = nt * 128
                                nc.gpsimd.dma_gather(
                                    out_ap=gsrc3[:, toff : toff + nt, :],
                                    in_ap=comb[k * CH : k * CH + rows, :],
                                    idxs_ap=ixt[:, ioff // 16 : (ioff + npos) // 16],
                                    num_idxs=npos,
                                    num_idxs_reg=npos,
                                    elem_size=128,
                                    single_packet=False,
                                    queue_num=qn % 4,
                                )
                                qn += 1
                                toff += nt
                                ioff += npos
                        if c.ADIND:
                            oads = wk.tile([128, TG], I32, tag="oads")
                            nc.sync.dma_start(
                                out=oads[:], in_=oad_t.ap()[:, gbase : gbase + TG]
                            )
                            nq = 4
                            step = (TG + nq - 1) // nq
                            for s0 in range(0, TG, step):
                                nt = min(step, TG - s0)
                                inst = nc.gpsimd.indirect_dma_start(
                                    out=ad3[:, s0 : s0 + nt, :],
                                    out_offset=None,
                                    in_=adl[:, :],
                                    in_offset=bass.IndirectOffsetOnAxis(
                                        ap=oads[:, s0 : s0 + nt], axis=0
                                    ),
                                )
                                if qn % 4:
                                    inst.queue = f"qPoolDynamic{qn % 4}"
                                qn += 1
                        else:
                            for s0 in range(0, TG, c.SUBT):
                                nt = min(c.SUBT, TG - s0)
                                npos = nt * 128
                                nc.gpsimd.dma_gather(
                                    out_ap=ad3[:, s0 : s0 + nt, :],
                                    in_ap=adl[:, :],
                                    idxs_ap=ixt[:, ioff // 16 : (ioff + npos) // 16],
                                    num_idxs=npos,
                                    num_idxs_reg=npos,
                                    elem_size=64,
                                    single_packet=False,
                                    queue_num=qn % 4,
                                )
                                qn += 1
                                ioff += npos
                        # f32 view of table rows
                        gsrc_f = gsrc[:].bitcast(F32).rearrange("p (t e) -> p t e", e=64)
                        gsrc_b = gsrc3
                        # e = a_src + a_dst
                        ew = wk.tile([128, TG * NH], F32, tag="ew")
                        ew3 = ew[:].rearrange("p (t h) -> p t h", h=NH)
                        nc.vector.tensor_tensor(
                            out=ew3[:],
                            in0=gsrc_f[:, :, AOFF : AOFF + NH],
                            in1=ad3[:, :, 0:NH],
                            op=mybir.AluOpType.add,
                        )
                        ew2 = wk.tile([128, TG * NH], F32, tag="ew2")
                        nc.vector.tensor_scalar_mul(out=ew2[:], in0=ew[:], scalar1=NEG_SLOPE)
                        nc.vector.tensor_tensor(
                            out=ew[:], in0=ew[:], in1=ew2[:], op=mybir.AluOpType.max
                        )
                        nc.scalar.activation(
                            out=ew[:], in_=ew[:], func=mybir.ActivationFunctionType.Exp
                        )
                        wb = wk.tile([128, TG * NH], BF16, tag="wb")
                        nc.vector.tensor_copy(out=wb[:], in_=ew[:])
                        wb3 = wb[:].rearrange("p (t h) -> p t h", h=NH)
                        # S = [w | w*h]
                        S = ssp.tile([128, TG * SW], BF16, tag="S")
                        S3 = S[:].rearrange("p (t e) -> p t e", e=SW)
                        nc.vector.tensor_copy(out=S3[:, :, 0:NH], in_=wb3[:, :, :])
                        if layer == 1:
                            nc.vector.tensor_tensor(
                                out=S3[:, :, NH:SW].rearrange(
                                    "p t (h ch) -> p t h ch", h=NH
                                ),
                                in0=gsrc_b[:, :, 0:DV].rearrange(
                                    "p t (h ch) -> p t h ch", h=NH
                                ),
                                in1=wb3[:, :, :].to_broadcast([128, TG, NH, c.C1]),
                                op=mybir.AluOpType.mult,
                            )
                        else:
                            nc.vector.tensor_tensor(
                                out=S3[:, :, NH:SW],
                                in0=gsrc_b[:, :, 0:DV],
                                in1=wb[:].to_broadcast([128, TG, DV]),
                                op=mybir.AluOpType.mult,
                            )
                        # group-batched aggregation + epilogue
                        nblk = len(ly.blocks)
                        b0 = ly.blocks[0]
                        aggG = psB.tile([128, nblk * SW], F32, tag="agg")
                        aggG3 = aggG[:].rearrange("p (b e) -> p b e", e=SW)
                        for bi, b in enumerate(ly.blocks):
                            runs = ly.runs[b]
                            ntot = sum(n for _, n in runs)
                            done = 0
                            for t0, nt in runs:
                                for j in range(nt):
                                    nc.tensor.matmul(
                                        out=aggG[:, bi * SW : (bi + 1) * SW],
                                        lhsT=mask[:, (t0 + j) * 128 : (t0 + j + 1) * 128],
                                        rhs=S3[:, t0 + j, :],
                                        start=(done == 0),
                                        stop=(done == ntot - 1),
                                    )
                                    done += 1
                        ssafe = wk.tile([128, nblk * NH], F32, tag="ssafe")
                        ssafe3 = ssafe[:].rearrange("p (b h) -> p b h", h=NH)
                        nc.vector.tensor_scalar_max(
                            out=ssafe3[:], in0=aggG3[:, :, 0:NH], scalar1=1e-16
                        )
                        rec = wk.tile([128, nblk * NH], F32, tag="rec")
                        nc.vector.reciprocal(out=rec[:], in_=ssafe[:])
                        rec3 = rec[:].rearrange("p (b h) -> p b h", h=NH)
                        o1G = wk.tile([128, nblk * DV], F32, tag="o1")
                        o1G3 = o1G[:].rearrange("p (b e) -> p b e", e=DV)
                        if layer == 1:
                            nc.vector.tensor_tensor(
                                out=o1G[:].rearrange(
                                    "p (b h ch) -> p b h ch", h=NH, ch=c.C1
                                ),
                                in0=aggG3[:, :, NH:SW].rearrange(
                                    "p b (h ch) -> p b h ch", h=NH
                                ),
                                in1=rec3[:, :, :].to_broadcast([128, nblk, NH, c.C1]),
                                op=mybir.AluOpType.mult,
                            )
                            nc.vector.tensor_tensor(
                                out=o1G3[:],
                                in0=o1G3[:],
                                in1=b1rep[:][:, None, :].to_broadcast([128, nblk, DV]),
                                op=mybir.AluOpType.add,
                            )
                            neg = wk.tile([128, nblk * DV], F32, tag="neg")
                            nc.vector.tensor_scalar_min(out=neg[:], in0=o1G[:], scalar1=0.0)
                            nc.scalar.activation(
                                out=neg[:], in_=neg[:],
                                func=mybir.ActivationFunctionType.Exp,
                            )
                            nc.vector.tensor_scalar_max(out=o1G[:], in0=o1G[:], scalar1=0.0)
                            nc.vector.tensor_tensor(
                                out=o1G[:], in0=o1G[:], in1=neg[:],
                                op=mybir.AluOpType.add,
                            )
                            nc.vector.tensor_scalar_add(out=o1G[:], in0=o1G[:], scalar1=-1.0)
                            h2G = psB.tile([128, nblk * (D2 + 2)], F32, tag="h2ps")
                            h2G3 = h2G[:].rearrange("p (b e) -> p b e", e=D2 + 2)
                            for bi in range(nblk):
                                eT_ps = psA.tile([DV, 128], F32, tag="pT")
                                nc.tensor.transpose(
                                    out=eT_ps[:],
                                    in_=o1G[:, bi * DV : (bi + 1) * DV],
                                    identity=ident[:],
                                )
                                eT = wk.tile([DV, 128], F32, tag="eT")
                                nc.vector.tensor_copy(out=eT[:], in_=eT_ps[:])
                                nc.tensor.matmul(
                                    out=h2G[:, bi * (D2 + 2) : (bi + 1) * (D2 + 2)],
                                    lhsT=eT[:], rhs=w2aug[:],
                                    start=True, stop=True,
                                )
                            s2G = wk.tile([128, nblk * 42], BF16, tag="s2")
                            s2G3 = s2G[:].rearrange("p (b e) -> p b e", e=42)
                            nc.vector.tensor_copy(
                                out=s2G3[:, :, 0:D2], in_=h2G3[:, :, 0:D2]
                            )
                            s2f3 = s2G[:].bitcast(F32).rearrange(
                                "p (b e) -> p b e", e=21
                            )
                            nc.vector.tensor_copy(
                                out=s2f3[:, :, 20:21], in_=h2G3[:, :, D2 : D2 + 1]
                            )
                            nc.sync.dma_start(
                                out=slab2[b0 * 128 : (b0 + nblk) * 128, 0:42]
                                .rearrange("(b p) e -> p b e", p=128),
                                in_=s2G3[:, :, :],
                            )
                            ad2G = wk.tile([128, nblk], F32, tag="ad2")
                            nc.vector.tensor_copy(
                                out=ad2G[:].rearrange("p (b e) -> p b e", e=1),
                                in_=h2G3[:, :, D2 + 1 : D2 + 2],
                            )
                            nc.sync.dma_start(
                                out=adl2[b0 * 128 : (b0 + nblk) * 128, 0:1]
                                .rearrange("(b p) e -> p b e", p=128),
                                in_=ad2G[:].rearrange("p (b e) -> p b e", e=1),
                            )
                        else:
                            nc.vector.tensor_tensor(
                                out=o1G3[:],
                                in0=aggG3[:, :, NH:SW],
                                in1=rec[:].to_broadcast([128, nblk, DV]),
                                op=mybir.AluOpType.mult,
                            )
                            nc.vector.tensor_tensor(
                                out=o1G3[:],
                                in0=o1G3[:],
                                in1=b2rep[:][:, None, :].to_broadcast([128, nblk, DV]),
                                op=mybir.AluOpType.add,
                            )
                            mxG = wk.tile([128, nblk], F32, tag="mx")
                            nc.vector.tensor_reduce(
                                out=mxG[:].rearrange("p (b e) -> p b e", e=1),
                                in_=o1G3[:],
                                axis=mybir.AxisListType.X,
                                op=mybir.AluOpType.max,
                            )
                            nc.vector.tensor_tensor(
                                out=o1G3[:],
                                in0=o1G3[:],
                                in1=mxG[:].to_broadcast([128, nblk, DV]),
                                op=mybir.AluOpType.subtract,
                            )
                            exG = wk.tile([128, nblk * DV], F32, tag="ex")
                            nc.scalar.activation(
                                out=exG[:], in_=o1G[:],
                                func=mybir.ActivationFunctionType.Exp,
                            )
                            smG = wk.tile([128, nblk], F32, tag="sm")
                            nc.vector.tensor_reduce(
                                out=smG[:].rearrange("p (b e) -> p b e", e=1),
                                in_=exG[:].rearrange("p (b e) -> p b e", e=DV),
                                axis=mybir.AxisListType.X,
                                op=mybir.AluOpType.add,
                            )
                            lgG = wk.tile([128, nblk], F32, tag="lg")
                            nc.scalar.activation(
                                out=lgG[:], in_=smG[:],
                                func=mybir.ActivationFunctionType.Ln,
                            )
                            nc.vector.tensor_tensor(
                                out=o1G3[:],
                                in0=o1G3[:],
                                in1=lgG[:].to_broadcast([128, nblk, DV]),
                                op=mybir.AluOpType.subtract,
                            )
                            nc.sync.dma_start(
                                out=out_t.ap()[b0 * 128 : (b0 + nblk) * 128, :]
                                .rearrange("(b p) e -> p b e", p=128),
                                in_=o1G3[:, :, :],
                            )
                        ipos += sum(ly.call_pos)
                        gbase += TG

                with nc.named_scope("edge1"):
                    edge_layer(1)
                with nc.named_scope("ag2"):
                    nc.gpsimd.collective_compute(
                        "AllGather",
                        mybir.AluOpType.bypass,
                        replica_groups=[list(range(c.NC))],
                        ins=[slab2[:, :].opt()],
                        outs=[comb2[:, :].opt()],
                    )
                with nc.named_scope("edge2"):
                    edge_layer(2)


# ------------------------------------------------------------------ driver


def make_runner(nc, n_cores=8):
    import jax
    from jax.sharding import Mesh, PartitionSpec
    from jax.experimental.shard_map import shard_map
    from concourse.bass2jax import (
        _bass_exec_p,
        install_neuronx_cc_hook,
        partition_id_tensor,
    )

    install_neuronx_cc_hook()
    partition_name = nc.partition_id_tensor.name if nc.partition_id_tensor else None

    in_names, out_names, out_avals, zero_outs = [], [], [], []
    for alloc in nc.m.functions[0].allocations:
        if not isinstance(alloc, mybir.MemoryLocationSet):
            continue
        name = alloc.memorylocations[0].name
        if alloc.kind == "ExternalInput":
            if name != partition_name:
                in_names.append(name)
        elif alloc.kind == "ExternalOutput":
            shape = tuple(alloc.tensor_shape)
            dtype = mybir.dt.np(alloc.dtype)
            out_names.append(name)
            out_avals.append(jax.core.ShapedArray(shape, dtype))
            zero_outs.append(np.zeros(shape, dtype))
    n_params = len(in_names)
    n_outs = len(out_avals)
    all_in_names = list(in_names) + list(out_names)
    if partition_name is not None:
        all_in_names.append(partition_name)

    donate = tuple(range(n_params, n_params + n_outs))

    def _body(*args):
        operands = list(args)
        if partition_name is not None:
            operands.append(partition_id_tensor())
        outs = _bass_exec_p.bind(
            *operands,
            out_avals=tuple(out_avals),
            in_names=tuple(all_in_names),
            out_names=tuple(out_names),
            lowering_input_output_aliases=(),
            sim_require_finite=True,
            sim_require_nnan=True,
            nc=nc,
        )
        return tuple(outs)

    devices = jax.devices()[:n_cores]
    mesh = Mesh(np.asarray(devices), ("core",))
    in_specs = (PartitionSpec("core"),) * (n_params + n_outs)
    out_specs = (PartitionSpec("core"),) * len(out_names)
    sharded = jax.jit(
        shard_map(
            _body, mesh=mesh, in_specs=in_specs, out_specs=out_specs, check_rep=False
        ),
        donate_argnums=donate,
        keep_unused=True,
    )

    def run(in_maps):
        per_core = [[np.asarray(m[name]) for name in in_names] for m in in_maps]
        concat_in = [
            np.concatenate([per_core[cc][i] for cc in range(n_cores)], axis=0)
            for i in range(n_params)
        ]
        concat_zeros = [
            np.zeros((n_cores * z.shape[0], *z.shape[1:]), z.dtype) for z in zero_outs
        ]
        out_arrs = sharded(*concat_in, *concat_zeros)
        jax.block_until_ready(out_arrs)
        return [
            {
                name: np.asarray(out_arrs[i]).reshape(n_cores, *out_avals[i].shape)[cc]
                for i, name in enumerate(out_names)
            }
            for cc in range(n_cores)
        ]

    return run


_CACHE = {}


def _get_runner(cfg: Cfg, plan: Plan, repeats: int = 1):
    key = (cfg, plan.key(), repeats)
    if key in _CACHE:
        return _CACHE[key]
    nc = bacc.Bacc(
        "TRN2",
        target_bir_lowering=False,
        debug=False,
        num_devices=cfg.NC,
        num_swdge_queues=4,
    )
    build(nc, cfg, plan, repeats)
    nc.compile()
    run = make_runner(nc, cfg.NC)
    _CACHE[key] = (run, nc)
    return _CACHE[key]


def kernel(
    x, edge_index, W1, att_src1, att_dst1, b1, W2, att_src2, att_dst2, b2, _cfg=None
):
    cfg = _cfg or Cfg()
    plan, in_maps = preprocess(x, edge_index, cfg)
    shared = {
        "W1": np.asarray(W1, np.float32),
        "att_src1": np.asarray(att_src1, np.float32),
        "att_dst1": np.asarray(att_dst1, np.float32),
        "b1": np.asarray(b1, np.float32),
        "W2": np.asarray(W2, np.float32),
        "att_src2": np.asarray(att_src2, np.float32),
        "att_dst2": np.asarray(att_dst2, np.float32),
        "b2": np.asarray(b2, np.float32),
    }
    for m in in_maps:
        m.update(shared)
    run, _nc = _get_runner(cfg, plan)
    res = run(in_maps)
    out = np.concatenate([r["out"][: cfg.NPC] for r in res], axis=0)
    return out.astype(np.float32)
